# revision 2
# baseline (speedup 1.0000x reference)
"""CrossViewRegionAligner Trainium2 kernel (v2: fp16 pairwise pipeline).

Computes the pairwise-MLP similarity logits on 8 NeuronCores (sat-axis
sharded, 192 rows/core), then performs the sequential greedy bipartite
matching on host (O(N^2) scalar work, inherently sequential).

v2 design (vs fp32 v1): fp32 matmuls cost 4 PE cycles/row on TRN2 while
fp16 costs 1, so the whole pairwise pipeline runs in fp16 with fp32 PSUM
accumulation.  The resulting logits carry ~1.9e-3 absolute error (vs the
min greedy runner-up gap of 6.3e-5), so the host greedy pass exactly
recomputes any row whose top-2 gap is below a 6e-3 safety threshold
(~130 rows) in fp64 -- picks are then exact.

The O((N+M) D^2) input projections (h_sat = sat@W1[:64]+b1 and
h_uav = uav@W1[64:]) are folded into the host-side prepack (0.2% of the
model FLOPs); the device kernel is the O(N M D) pairwise part:

  h1(i,j)  = relu(c_i + hu_j)         DVE tensor_scalar fused add+max,
             fp16 [128,1536] per 2 sat rows (per-partition scalar =
             packed fp32 c pair; 2-row-replicated hu)
  h2(i,j)  = relu(blockdiag(W2,W2)^T h1 + b2)    PE fp16 K=128 -> PSUM,
             ACT fused relu+bias PSUM->SBUF fp16, 4 rows/instr
  logit    = blockdiag(W3 x4)^T h2    PE fp16, 8-pass PSUM accumulation
             via shifted zero-padded weight variants (32 logit rows per
             PSUM tile), DVE copies PSUM->SBUF, one tail DMA
"""

import os

import numpy as np

# If the axon NTFF profile hook is unavailable, a BASS_TRACE=1 environment
# would crash run_bass_kernel_spmd with ModuleNotFoundError -- disable
# tracing only in that case.
try:
    from antenv import axon_hooks as _axon_hooks  # noqa: F401
except Exception:
    os.environ.setdefault("BASS_NEVER_TRACE", "1")

import concourse.bass as bass
import concourse.bacc as bacc
import concourse.mybir as mybir
from concourse.tile import TileContext, add_dep_helper
from concourse.bass_utils import run_bass_kernel_spmd

FP32 = mybir.dt.float32
FP16 = mybir.dt.float16
N_SAT = 1536
N_UAV = 1536
D = 64
N_CORES = 8
RPC = N_SAT // N_CORES          # sat rows per core = 192
PASSES = RPC // 4               # 4 sat rows per pass = 48
GROUP = 8                       # passes per logit-accumulation group (32 rows)
N_GROUPS = PASSES // GROUP      # 6
CHUNK = 512                     # matmul free-dim chunk (one PSUM bank, fp32)
N_CHUNKS = N_UAV // CHUNK       # 3
H1S = 4                         # h1 ring slots (per a/b buffer)
H2S = 10                        # h2 ring slots
DELTA = 6e-3                    # greedy fixup threshold (max device logit
                                # err measured 1.9e-3; 3x safety margin)

# fp16 blob layout (columns)
OFF_HU = 0                          # huav 2x-replicated [128, 1536]
OFF_W2P = OFF_HU + N_UAV            # blockdiag(W2,W2) [128, 64]
OFF_W3P = OFF_W2P + 64              # 8 shifted blockdiag(W3 x4) [128, 256]
BLOB16_W = OFF_W3P + GROUP * 32     # 1856

# fp32 blob layout (columns)
OFF_CP = 0                          # c_pack [128, 96]
OFF_B2P = OFF_CP + RPC // 2         # tile(b2, 4) [128, 1]
BLOB32_W = OFF_B2P + 1              # 97

_CACHED_NC = None
LAST_RESULT = None  # BassKernelResults of the most recent run (for profiling)


def _build_nc():
    nc = bacc.Bacc(trn_type="TRN2")

    blob16 = nc.dram_tensor("blob16", [128, BLOB16_W], FP16, kind="ExternalInput")
    blob32 = nc.dram_tensor("blob32", [128, BLOB32_W], FP32, kind="ExternalInput")
    lout_all = nc.dram_tensor(
        "lout_all", [GROUP * 4, N_GROUPS * N_CHUNKS * CHUNK], FP32,
        kind="ExternalOutput",
    )

    with TileContext(nc) as tc:
        _body(nc, tc, blob16, blob32, lout_all)
    nc.finalize()
    return nc


def _body(nc, tc, blob16, blob32, lout_all):
    from contextlib import ExitStack

    with ExitStack() as ctx:
        consts = ctx.enter_context(tc.tile_pool(name="consts", bufs=1))
        psum = ctx.enter_context(tc.tile_pool(name="psum", bufs=1, space="PSUM"))

        # ---- load constants as TWO blobs (one DMA queue sem each) --------
        blob16_sb = consts.tile([128, BLOB16_W], FP16, tag="blob16")
        nc.gpsimd.dma_start(blob16_sb[:], blob16[:])
        blob32_sb = consts.tile([128, BLOB32_W], FP32, tag="blob32")
        nc.gpsimd.dma_start(blob32_sb[:], blob32[:])

        huav_sb = blob16_sb[0:128, OFF_HU : OFF_HU + N_UAV]
        w2p_sb = blob16_sb[0:128, OFF_W2P : OFF_W2P + 64]
        w3p_sb = blob16_sb[0:128, OFF_W3P : OFF_W3P + GROUP * 32]
        c_pack = blob32_sb[0:128, OFF_CP : OFF_CP + RPC // 2]
        b2p_sb = blob32_sb[0:128, OFF_B2P : OFF_B2P + 1]

        # Permanent PSUM tiles (no pool recycling => no slot-transition
        # multi-waits; same-engine WAW is program order). 3+3+1+1 = 8 banks.
        psA = psum.tile([128, N_UAV], FP32, tag="psA")
        psB = psum.tile([128, N_UAV], FP32, tag="psB")
        lpA = psum.tile([GROUP * 4, CHUNK], FP32, tag="lpA")
        lpB = psum.tile([GROUP * 4, CHUNK], FP32, tag="lpB")

        # Permanent SBUF rings
        h1A = consts.tile([128, H1S * N_UAV], FP16, tag="h1A")
        h1B = consts.tile([128, H1S * N_UAV], FP16, tag="h1B")
        h2buf = consts.tile([128, H2S * N_UAV], FP16, tag="h2buf")
        lsb_all = consts.tile(
            [GROUP * 4, N_GROUPS * N_CHUNKS * CHUNK], FP32, tag="lsb_all"
        )
        scratch16 = consts.tile([128, 2], FP16, tag="scratch16")
        scratch32 = consts.tile([128, 1], FP32, tag="scratch32")

        # ---- per-engine DMA-sem consumption probes -----------------------
        # (each ISA instruction holds at most ONE sem wait; consume the two
        # input-DMA queue sems once per engine that reads from the blobs)
        w2c = w2p_sb[:, 0:1]
        # PE reads blob16 (w2p, w3p): probe matmul into lpA (overwritten by
        # the first start=True L3 matmul much later).
        nc.tensor.matmul(lpA[0:1, 0:1], w2c, w2c, skip_group_check=True)
        # DVE reads blob16 (huav) + blob32 (c_pack scalars): two probes.
        nc.vector.tensor_copy(scratch16[:, 0:1], huav_sb[:, 0:1])
        nc.vector.tensor_copy(scratch32[:], c_pack[:, 0:1])
        # ACT reads blob32 (b2p bias).
        nc.scalar.copy(scratch32[:], b2p_sb[:])

        prev = {}  # last emitted instruction per engine, for chain edges

        def chain(key, binst):
            if key in prev:
                add_dep_helper(binst.ins, prev[key].ins, sync=False, reason="chain")
            prev[key] = binst
            return binst

        # ---- main loop ---------------------------------------------------
        # Per-pass single-sem wait discipline:
        #   DVE ts-a/ts-b: {PE tick of pass t-4's matmuls} (h1 slot WAR)
        #   PE opener:     {ACT tick of pass t-2} (psum WAW vs ACT read)
        #   PE mm x6:      {DVE ticks} one new value each (h1A/h1B RAW)
        #   ACT act:       {PE tick} (psum RAW; h2 slot WAR is an earlier
        #                  PE tick, covered monotonically)
        #   PE L3 opener:  {DVE tick} (lp WAW vs DVE's PSUM->SBUF copy)
        #   PE L3 mm x8:   {ACT tick} (h2buf RAW)
        #   DVE lsb copy:  {PE tick of stop matmul}
        for g in range(N_GROUPS):
            for q in range(GROUP):
                t = g * GROUP + q  # pass index; sat rows 4t..4t+3
                h1o = (t % H1S) * N_UAV
                h2o = (t % H2S) * N_UAV
                # --- DVE: h1 = relu(huav + c), fp16, 2 sat rows/instr ---
                chain("v", nc.vector.tensor_scalar(
                    out=h1A[:, h1o : h1o + N_UAV],
                    in0=huav_sb[:],
                    scalar1=c_pack[:, 2 * t : 2 * t + 1],
                    scalar2=0.0,
                    op0=mybir.AluOpType.add,
                    op1=mybir.AluOpType.max,
                ))
                chain("v", nc.vector.tensor_scalar(
                    out=h1B[:, h1o : h1o + N_UAV],
                    in0=huav_sb[:],
                    scalar1=c_pack[:, 2 * t + 1 : 2 * t + 2],
                    scalar2=0.0,
                    op0=mybir.AluOpType.add,
                    op1=mybir.AluOpType.max,
                ))
                # --- PE: opener + L2 matmuls ---
                ps = psA if t % 2 == 0 else psB
                chain("p", nc.tensor.matmul(
                    ps[0:1, 0:1], w2c, w2c, skip_group_check=True
                ))
                for c in range(N_CHUNKS):
                    sl = slice(c * CHUNK, (c + 1) * CHUNK)
                    hslc = slice(h1o + c * CHUNK, h1o + (c + 1) * CHUNK)
                    chain("p", nc.tensor.matmul(ps[0:64, sl], w2p_sb[:], h1A[:, hslc]))
                    chain("p", nc.tensor.matmul(ps[64:128, sl], w2p_sb[:], h1B[:, hslc]))
                # --- ACT: h2 = relu(ps + b2) -> fp16 SBUF, 4 rows/instr ---
                chain("a", nc.scalar.activation(
                    h2buf[:, h2o : h2o + N_UAV],
                    ps[:],
                    mybir.ActivationFunctionType.Relu,
                    bias=b2p_sb[:],
                ))

            # --- L3: accumulate GROUP passes into one PSUM tile ---
            for c in range(N_CHUNKS):
                sl = slice(c * CHUNK, (c + 1) * CHUNK)
                n = g * N_CHUNKS + c
                lp = lpA if n % 2 == 0 else lpB
                # opener carries the DVE wait (lp WAW vs PSUM->SBUF copy)
                chain("p", nc.tensor.matmul(
                    lp[0:1, 0:1], w2c, w2c, skip_group_check=True
                ))
                for q in range(GROUP):
                    p = g * GROUP + q
                    ho = (p % H2S) * N_UAV + c * CHUNK
                    chain("p", nc.tensor.matmul(
                        lp[:],
                        w3p_sb[:, q * 32 : (q + 1) * 32],
                        h2buf[:, ho : ho + CHUNK],
                        start=(q == 0),
                        stop=(q == GROUP - 1),
                    ))
                chain("v", nc.vector.tensor_copy(
                    lsb_all[:, n * CHUNK : (n + 1) * CHUNK], lp[:]
                ))

        # single tail DMA of all logits (waits only DVE's last copy)
        nc.sync.dma_start(lout_all[:], lsb_all[:])


def _prepack(sat_shard, uav_regions, W1, b1, W2, b2, W3):
    f32, f16 = np.float32, np.float16
    W1a, W1b = W1[:D], W1[D:]
    hs = (sat_shard.astype(f32) @ W1a.astype(f32) + b1.astype(f32))  # [192, 64]
    hu = (uav_regions.astype(f32) @ W1b.astype(f32))                 # [1536, 64]

    blob16 = np.zeros((128, BLOB16_W), f16)
    hu16 = hu.astype(f16).T                 # [64, 1536]
    blob16[0:D, OFF_HU : OFF_HU + N_UAV] = hu16
    blob16[D:128, OFF_HU : OFF_HU + N_UAV] = hu16
    blob16[0:D, OFF_W2P : OFF_W2P + 32] = W2
    blob16[D:128, OFF_W2P + 32 : OFF_W2P + 64] = W2
    # variant q places blockdiag(W3 x4) rows at output columns 4q..4q+3
    for q in range(GROUP):
        for r in range(4):
            blob16[32 * r : 32 * (r + 1), OFF_W3P + q * 32 + 4 * q + r] = W3[:, 0]

    blob32 = np.zeros((128, BLOB32_W), f32)
    # c_pack col t = (hs[2t] | hs[2t+1]); pass t uses cols 2t (rows 4t,4t+1)
    # and 2t+1 (rows 4t+2,4t+3)
    cp = blob32[:, OFF_CP : OFF_CP + RPC // 2]
    cp[0:D, :] = hs[0::2].T
    cp[D:128, :] = hs[1::2].T
    blob32[:, OFF_B2P] = np.tile(b2, 4)
    return dict(
        blob16=np.ascontiguousarray(blob16),
        blob32=np.ascontiguousarray(blob32),
    )


def _greedy_assign(sim, sat_regions, uav_regions, W1, b1, W2, b2, W3):
    """Sequential greedy matching identical to the reference scan, with
    exact fp64 recompute of rows whose fast-path top-2 gap is < DELTA."""
    f64 = np.float64
    hs64 = sat_regions.astype(f64) @ W1[:D].astype(f64) + b1.astype(f64)
    hu64 = uav_regions.astype(f64) @ W1[D:].astype(f64)
    W2_64, b2_64, W3_64 = W2.astype(f64), b2.astype(f64), W3.astype(f64)

    def exact_row(i):
        h1 = np.maximum(hs64[i][None, :] + hu64, 0.0)     # [M, 64]
        h2 = np.maximum(h1 @ W2_64 + b2_64, 0.0)          # [M, 32]
        return (h2 @ W3_64)[:, 0]                         # [M]

    scores = sim.astype(np.float32).copy()
    assign = np.empty(N_SAT, np.int64)
    mask = np.ones(N_UAV, bool)
    n_fixed = 0
    for i in range(N_SAT):
        row = np.where(mask, scores[i], -np.inf)
        j = int(np.argmax(row))
        if i < N_SAT - 1:
            v1 = row[j]
            row[j] = -np.inf
            if v1 - row.max() < DELTA:
                er = np.where(mask, exact_row(i), -np.inf)
                j = int(np.argmax(er))
                n_fixed += 1
        assign[i] = j
        mask[j] = False
    return assign


def kernel(sat_regions, uav_regions, W1, b1, W2, b2, W3, b3):
    global _CACHED_NC
    if _CACHED_NC is None:
        _CACHED_NC = _build_nc()
    nc = _CACHED_NC

    in_maps = []
    for k in range(N_CORES):
        shard = sat_regions[k * RPC : (k + 1) * RPC]
        in_maps.append(_prepack(shard, uav_regions, W1, b1, W2, b2, W3))

    res = run_bass_kernel_spmd(nc, in_maps, core_ids=list(range(N_CORES)))
    global LAST_RESULT
    LAST_RESULT = res
    sim = np.empty((N_SAT, N_UAV), np.float32)
    for k in range(N_CORES):
        la = res.results[k]["lout_all"]  # [32, 18*512]
        for n in range(N_GROUPS * N_CHUNKS):
            g, c = divmod(n, N_CHUNKS)
            sim[
                k * RPC + g * GROUP * 4 : k * RPC + (g + 1) * GROUP * 4,
                c * CHUNK : (c + 1) * CHUNK,
            ] = la[:, n * CHUNK : (n + 1) * CHUNK]

    assign = _greedy_assign(sim, sat_regions, uav_regions, W1, b1, W2, b2, W3)
    out = np.stack([sat_regions, uav_regions[assign]], axis=1)
    return np.ascontiguousarray(out, dtype=np.float32)
